# revision 12
# baseline (speedup 1.0000x reference)
"""Tree-GRU (arity-8, depth-5) over embedded leaves on 8 TRN2 NeuronCores.

Sharding: data-parallel over subtrees. Each core takes 4096 contiguous leaves
and runs levels 5 and 4 of the tree locally (512 -> 64 parents). The last two
per-core levels (64 -> 8 -> 1) and the root are small latency-bound GRU
cascades (free dim <= 8) done on host in fp64 after gathering the per-core
level-4 outputs, extending the baseline's host-side root reduction.

Device layout is feature-transposed: tensors live as [128 part, 3 ktile, ...]
with feature f = 128*k + p, so the GRU matmuls contract the partition dim.

Embeddings arrive per GRU step: tokens are host-permuted child-major, each
child's 512 rows fetched by 4 indirect DMAs (leaf-major) and flipped
feature-major by 4 xbar transpose-DMAs on the HWDGE rings — no tensor-engine
transposes, no PSUM, and the first GRU matmul can start after ~2 gathers.

Level 512 keeps one PSUM bank per (role, jo) output tile at N=512; each step
emits gi matmuls of units j0/j1 ahead of any hh matmul so the tensor engine
holds ~3.8us of h-independent work to hide the previous step's gate chain.
Unit j1 owns 4 banks (double-buffered step to step); j0 and j2 share the
other 4, with j2's allocation waiting on j0's progressively-freed banks
behind hh j1. Biases ride the scalar-activation bias port. The per-step
output accumulator is kept child-major (vector engine) so the final step
writes level 4's input directly as a fused raw-sum add; the 1/8 output-mean
scale is folded into a pre-scaled copy of W_ih used by level 4. Level 64
injects biases into PSUM via a K=3 one-hot matmul (the only start=True
write), collapsing the gate chain to jo-spanning instructions.
"""

import numpy as np
import ml_dtypes

ARITY = 8
DIM = 384
VOCAB = 32000
NCORES = 8
P = 128
J = 3  # DIM // 128 feature tiles
N_LEAVES = 32768
LEAVES_CORE = N_LEAVES // NCORES  # 4096
P5 = LEAVES_CORE // ARITY  # 512 level-5 parents per core
P4 = P5 // ARITY  # 64 level-4 parents per core
GT = P5 // P  # 4 gather tiles per child

BF16 = ml_dtypes.bfloat16

_PROG_CACHE = {}


def _emit(tc, nc, aps):
    import concourse.mybir as mybir
    import concourse.bass as bass

    f32 = mybir.dt.float32
    bf16 = mybir.dt.bfloat16
    Sig = mybir.ActivationFunctionType.Sigmoid
    Tanh = mybir.ActivationFunctionType.Tanh
    Add = mybir.AluOpType.add
    Sub = mybir.AluOpType.subtract
    Mult = mybir.AluOpType.mult

    tokens, embed, wih_t, wih_s, whh_t, biases, biases_mm, onehot3, out_hacc, out_hf = aps

    from contextlib import ExitStack

    with ExitStack() as ctx:
        const = ctx.enter_context(tc.tile_pool(name="const", bufs=1))
        xpool = ctx.enter_context(tc.tile_pool(name="xpool", bufs=1))
        gpool = ctx.enter_context(tc.tile_pool(name="gpool", bufs=3))
        state = ctx.enter_context(tc.tile_pool(name="state", bufs=1))
        gates = ctx.enter_context(tc.tile_pool(name="gates", bufs=4))
        pspool = ctx.enter_context(tc.tile_pool(name="pspool", bufs=4, space="PSUM"))
        pspool2 = ctx.enter_context(tc.tile_pool(name="pspool2", bufs=4, space="PSUM"))

        # ---- tokens first (sync ring), weights on the scalar ring so neither
        # blocks the gather->transpose chain on the sync ring ----
        tok_sb = const.tile([P, ARITY * GT], mybir.dt.int32)
        nc.sync.dma_start(tok_sb[:], tokens.rearrange("(c g p) -> p (c g)", p=P, g=GT))

        wih_sb = const.tile([P, J, 9, P], bf16)
        wih_s_sb = const.tile([P, J, 9, P], bf16)
        whh_sb = const.tile([P, J, 9, P], bf16)
        bias_sb = const.tile([P, 12], f32)
        bias3_sb = const.tile([3, 4, P], bf16)
        onehot3_sb = const.tile([3, 3, 512], bf16)
        nc.scalar.dma_start(wih_sb[:], wih_t[:])
        nc.scalar.dma_start(whh_sb[:], whh_t[:])
        nc.scalar.dma_start(wih_s_sb[:], wih_s[:])
        nc.scalar.dma_start(bias_sb[:], biases[:])
        nc.scalar.dma_start(bias3_sb[:], biases_mm[:])
        nc.scalar.dma_start(onehot3_sb[:], onehot3[:])

        # ---- per-child gather (leaf-major) + xbar transpose to feature-major ----
        x5 = xpool.tile([P, ARITY, J, P5], bf16, name="x5", tag="x5")
        for t in range(ARITY):
            c = ARITY - 1 - t  # children consumed in reverse: child 7 first
            xg = gpool.tile([P, GT, DIM], bf16, name="xg", tag="xg")
            for g in range(GT):
                gi_inst = nc.gpsimd.indirect_dma_start(
                    out=xg[:, g, :],
                    out_offset=None,
                    in_=embed[:],
                    in_offset=bass.IndirectOffsetOnAxis(
                        ap=tok_sb[:, c * GT + g : c * GT + g + 1], axis=0
                    ),
                )
                if g % 2 == 1:
                    gi_inst.ins.queue = "qPoolDynamic1"
            for g in range(GT):
                nc.sync.dma_start_transpose(
                    x5[:, c, :, g * P : (g + 1) * P], xg[:, g, :]
                )

        x4 = xpool.tile([P, ARITY, J, P4], bf16, name="x4", tag="x4")

        def psum_tile(jo):
            # 8 banks for 12 role-tiles per step: unit j1 owns pspool (reuse
            # waits on the previous step's j1 gates); j0/j2 share pspool2 —
            # j2 waits on same-step j0 gates (freed progressively under hh
            # j1), j0 on the previous step's j2 gates. All waits point at
            # strictly earlier FIFO positions: no deadlock.
            if jo == 1:
                return pspool.tile([P, 512], f32, name="ps", tag="ps")
            return pspool2.tile([P, 512], f32, name="ps2", tag="ps2")

        # =================== level 5: 512 parents, leaf children ===================
        h5 = state.tile([P, J, P5], bf16, name="h5", tag="h5")
        hacc5 = state.tile([P, J, ARITY, P4], f32, name="hacc5", tag="hacc5")
        nc.gpsimd.memset(hacc5[:], 0.0)
        csum5 = state.tile([P, J, P4], f32, name="csum5", tag="csum5")

        with nc.named_scope("level_512"):
            for t in range(ARITY):
                c = ARITY - 1 - t
                leaf0 = t == 0

                ps_r = [None] * J
                ps_z = [None] * J
                ps_in = [None] * J
                ps_hn = [None] * J

                def emit_gi(jo):
                    ps_r[jo] = psum_tile(jo)
                    ps_z[jo] = psum_tile(jo)
                    ps_in[jo] = psum_tile(jo)
                    if not leaf0:
                        ps_hn[jo] = psum_tile(jo)
                    for ps, moff in ((ps_r[jo], 0), (ps_z[jo], 3), (ps_in[jo], 6)):
                        for ji in range(J):
                            nc.tensor.matmul(
                                ps[:, :P5],
                                wih_sb[:, ji, moff + jo, :],
                                x5[:, c, ji, :],
                                start=(ji == 0),
                                stop=(ji == 2 and (moff == 6 or leaf0)),
                            )

                def emit_hh(jo):
                    if leaf0:
                        return
                    for ps, moff in ((ps_r[jo], 0), (ps_z[jo], 3), (ps_hn[jo], 6)):
                        for ji in range(J):
                            nc.tensor.matmul(
                                ps[:, :P5],
                                whh_sb[:, ji, moff + jo, :],
                                h5[:, ji, :],
                                start=(ji == 0 and moff == 6),
                                stop=(ji == 2),
                            )

                emit_gi(0)
                emit_gi(1)
                emit_hh(0)
                emit_hh(1)
                emit_gi(2)
                emit_hh(2)

                for jo in range(J):
                    r_sb = gates.tile([P, P5], bf16, name="r_sb", tag="r_sb")
                    z_sb = gates.tile([P, P5], bf16, name="z_sb", tag="z_sb")
                    n_sb = gates.tile([P, P5], bf16, name="n_sb", tag="n_sb")
                    rhn = gates.tile([P, P5], f32, name="rhn", tag="rhn")
                    t1 = gates.tile([P, P5], bf16, name="t1", tag="t1")

                    nc.scalar.activation(
                        r_sb[:], ps_r[jo][:, :P5], Sig, bias=bias_sb[:, jo : jo + 1]
                    )
                    nc.scalar.activation(
                        z_sb[:], ps_z[jo][:, :P5], Sig, bias=bias_sb[:, 3 + jo : 4 + jo]
                    )
                    if leaf0:
                        nc.vector.tensor_scalar_mul(
                            rhn[:], r_sb[:], bias_sb[:, 6 + jo : 7 + jo]
                        )
                    else:
                        nc.vector.scalar_tensor_tensor(
                            out=rhn[:],
                            in0=ps_hn[jo][:, :P5],
                            scalar=bias_sb[:, 6 + jo : 7 + jo],
                            in1=r_sb[:],
                            op0=Add,
                            op1=Mult,
                        )
                    nc.vector.tensor_tensor(
                        out=rhn[:], in0=rhn[:], in1=ps_in[jo][:, :P5], op=Add
                    )
                    nc.scalar.activation(
                        n_sb[:], rhn[:], Tanh, bias=bias_sb[:, 9 + jo : 10 + jo]
                    )
                    hsl = h5[:, jo, :]
                    if leaf0:
                        nc.vector.tensor_tensor(out=t1[:], in0=z_sb[:], in1=n_sb[:], op=Mult)
                        nc.vector.tensor_tensor(out=hsl, in0=n_sb[:], in1=t1[:], op=Sub)
                    else:
                        nc.vector.tensor_tensor(out=t1[:], in0=hsl, in1=n_sb[:], op=Sub)
                        nc.vector.tensor_tensor(out=t1[:], in0=z_sb[:], in1=t1[:], op=Mult)
                        nc.vector.tensor_tensor(out=hsl, in0=n_sb[:], in1=t1[:], op=Add)

                # output accumulation after the chain ops so it never delays
                # the next step's recurrent matmuls
                for jo in range(J):
                    hsl = h5[:, jo, :]
                    hperm = hsl.rearrange("p (q c) -> p c q", c=ARITY)
                    if t == ARITY - 1:
                        nc.vector.tensor_reduce(
                            out=csum5[:, jo, :],
                            in_=hsl.rearrange("p (q c) -> p q c", c=ARITY),
                            axis=mybir.AxisListType.X,
                            op=Add,
                        )
                        nc.vector.tensor_tensor(
                            out=x4[:, :, jo, :],
                            in0=hacc5[:, jo],
                            in1=hperm,
                            op=Add,
                        )
                    else:
                        nc.vector.tensor_tensor(
                            out=hacc5[:, jo], in0=hacc5[:, jo], in1=hperm, op=Add
                        )

        # =================== level 4: 64 parents ===================
        h4 = state.tile([P, J, P4], bf16, name="h4", tag="h4")
        nc.scalar.mul(h4[:], csum5[:], 1.0 / ARITY)
        hacc4 = state.tile([P, J, P4], f32, name="hacc4", tag="hacc4")
        nc.gpsimd.memset(hacc4[:], 0.0)
        N3 = J * P4  # 192

        with nc.named_scope("level_64"):
            for t in range(ARITY):
                c = ARITY - 1 - t
                pool = pspool if t % 2 == 0 else pspool2
                tag = "ps" if t % 2 == 0 else "ps2"
                ps_r, ps_z, ps_hn, ps_in = (
                    pool.tile([P, 512], f32, name=tag, tag=tag) for _ in range(4)
                )

                def view3(pst):
                    return pst[:, :N3].rearrange("p (j n) -> p j n", j=3)

                # bias matmul: the only start=True write, covering each tile
                for pst, ro in ((ps_r, 0), (ps_z, 1), (ps_hn, 2), (ps_in, 3)):
                    nc.tensor.matmul(
                        pst[:, :N3],
                        bias3_sb[:, ro, :],
                        onehot3_sb[:, :, :P4],
                        start=True,
                        stop=False,
                    )
                for pst, moff in ((ps_r, 0), (ps_z, 3), (ps_in, 6)):
                    for jo in range(J):
                        for ji in range(J):
                            nc.tensor.matmul(
                                pst[:, jo * P4 : (jo + 1) * P4],
                                wih_s_sb[:, ji, moff + jo, :],
                                x4[:, c, ji, :],
                                start=False,
                                stop=(moff == 6 and jo == 2 and ji == 2),
                            )
                for pst, moff in ((ps_r, 0), (ps_z, 3), (ps_hn, 6)):
                    for jo in range(J):
                        for ji in range(J):
                            nc.tensor.matmul(
                                pst[:, jo * P4 : (jo + 1) * P4],
                                whh_sb[:, ji, moff + jo, :],
                                h4[:, ji, :],
                                start=False,
                                stop=(jo == 2 and ji == 2),
                            )

                r_sb = gates.tile([P, J, P4], bf16, name="r4", tag="r4")
                z_sb = gates.tile([P, J, P4], bf16, name="z4", tag="z4")
                n_sb = gates.tile([P, J, P4], bf16, name="n4", tag="n4")
                rhn = gates.tile([P, J, P4], f32, name="rhn4", tag="rhn4")
                t1 = gates.tile([P, J, P4], bf16, name="t14", tag="t14")

                nc.scalar.activation(r_sb[:], view3(ps_r), Sig)
                nc.scalar.activation(z_sb[:], view3(ps_z), Sig)
                nc.vector.tensor_tensor(
                    out=rhn[:], in0=view3(ps_hn), in1=r_sb[:], op=Mult
                )
                nc.vector.tensor_tensor(
                    out=rhn[:], in0=rhn[:], in1=view3(ps_in), op=Add
                )
                nc.scalar.activation(n_sb[:], rhn[:], Tanh)
                nc.vector.tensor_tensor(out=t1[:], in0=h4[:], in1=n_sb[:], op=Sub)
                nc.vector.tensor_tensor(out=t1[:], in0=z_sb[:], in1=t1[:], op=Mult)
                nc.vector.tensor_tensor(out=h4[:], in0=n_sb[:], in1=t1[:], op=Add)
                nc.vector.tensor_tensor(
                    out=hacc4[:], in0=hacc4[:], in1=h4[:], op=Add
                )

        # ---- outputs: raw h-sum (x3*8) and final hiddens of the 64 nodes ----
        nc.sync.dma_start(out_hacc[:], hacc4[:])
        nc.sync.dma_start(out_hf[:], h4[:])


def _build_program():
    if "prog" in _PROG_CACHE:
        return _PROG_CACHE["prog"]
    import concourse.bacc as bacc
    import concourse.mybir as mybir
    import concourse.tile as tile

    f32 = mybir.dt.float32
    bf16 = mybir.dt.bfloat16

    nc = bacc.Bacc(
        "TRN2",
        target_bir_lowering=False,
        debug=False,
        enable_asserts=False,
        num_devices=NCORES,
        num_swdge_queues=2,
    )
    tokens = nc.dram_tensor(
        "tokens", [LEAVES_CORE], mybir.dt.int32, kind="ExternalInput"
    ).ap()
    embed = nc.dram_tensor("embed", [VOCAB, DIM], bf16, kind="ExternalInput").ap()
    wih_t = nc.dram_tensor("wih_t", [P, J, 9, P], bf16, kind="ExternalInput").ap()
    wih_s = nc.dram_tensor("wih_s", [P, J, 9, P], bf16, kind="ExternalInput").ap()
    whh_t = nc.dram_tensor("whh_t", [P, J, 9, P], bf16, kind="ExternalInput").ap()
    biases = nc.dram_tensor("biases", [P, 12], f32, kind="ExternalInput").ap()
    biases_mm = nc.dram_tensor("biases_mm", [3, 4, P], bf16, kind="ExternalInput").ap()
    onehot3 = nc.dram_tensor("onehot3", [3, 3, 512], bf16, kind="ExternalInput").ap()
    out_hacc = nc.dram_tensor("out_hacc", [P, J, P4], f32, kind="ExternalOutput").ap()
    out_hf = nc.dram_tensor("out_hf", [P, J, P4], bf16, kind="ExternalOutput").ap()

    with tile.TileContext(nc) as tc:
        _emit(
            tc,
            nc,
            (
                tokens,
                embed,
                wih_t,
                wih_s,
                whh_t,
                biases,
                biases_mm,
                onehot3,
                out_hacc,
                out_hf,
            ),
        )
    nc.compile()
    _PROG_CACHE["prog"] = nc
    return nc


def _retile_weights(w):
    # w: [1152, 384] -> lhsT tiles [128(k_part), 3(k), 9(m), 128(m_col)] bf16
    wt = np.ascontiguousarray(w.T)  # [384, 1152]
    wt = wt.reshape(J, P, 9, P).transpose(1, 0, 2, 3)
    return np.ascontiguousarray(wt).astype(BF16)


def _prep_bias(b_ih, b_hh):
    biases = np.zeros((P, 12), np.float32)
    comb = (b_ih + b_hh).reshape(9, P)
    biases[:, 0:6] = comb[0:6].T
    biases[:, 6:9] = b_hh.reshape(9, P)[6:9].T
    biases[:, 9:12] = b_ih.reshape(9, P)[6:9].T
    return biases


def _prep_bias_mm(b_ih, b_hh):
    # lhsT[k, ro, q] = bias[q, 3*ro + k]: the K=3 bias matmul against the
    # one-hot rhs yields out[q, (j, n)] = bias[q, 3*ro + j].
    b = _prep_bias(b_ih, b_hh)  # [128, 12] cols: r0..2 z0..2 hn0..2 in0..2
    out = b.T.reshape(4, 3, P).transpose(1, 0, 2)
    return np.ascontiguousarray(out).astype(BF16)


def _prep_onehot3():
    out = np.zeros((3, 3, 512), np.float32)
    for k in range(3):
        out[k, k, :] = 1.0
    return out.astype(BF16)


def _prep_tokens(tokens_core):
    # child-major: slot (c, g, p) holds tokens[(g*128 + p)*8 + c]
    tok = tokens_core.reshape(P5, ARITY).T  # [8 child, 512 parent]
    return np.ascontiguousarray(tok.reshape(ARITY * GT * P))


def _gru_level(x_children, h0, w_ih, w_hh, b_ih, b_hh):
    # x_children: [A, N, D] in original child order; consumed reversed.
    h = h0
    acc = np.zeros_like(h)
    for t in range(ARITY):
        x_t = x_children[ARITY - 1 - t]
        gi = x_t @ w_ih.T + b_ih
        gh = h @ w_hh.T + b_hh
        i_r, i_z, i_n = np.split(gi, 3, axis=-1)
        h_r, h_z, h_n = np.split(gh, 3, axis=-1)
        r = 1.0 / (1.0 + np.exp(-(i_r + h_r)))
        z = 1.0 / (1.0 + np.exp(-(i_z + h_z)))
        n = np.tanh(i_n + r * h_n)
        h = (1.0 - z) * n + z * h
        acc += h
    return acc / ARITY, h


def kernel(leaf_tokens, embed_table, w_ih, w_hh, b_ih, b_hh):
    from concourse.bass_utils import run_bass_kernel_spmd

    leaf_tokens = np.asarray(leaf_tokens, np.int32)
    embed_table = np.asarray(embed_table, np.float32)
    w_ih = np.asarray(w_ih, np.float32)
    w_hh = np.asarray(w_hh, np.float32)
    b_ih = np.asarray(b_ih, np.float32)
    b_hh = np.asarray(b_hh, np.float32)

    nc = _build_program()

    embed_bf = embed_table.astype(BF16)
    wih_t = _retile_weights(w_ih)
    wih_s = _retile_weights(w_ih / ARITY)
    whh_t = _retile_weights(w_hh)
    biases = _prep_bias(b_ih, b_hh)
    biases_mm = _prep_bias_mm(b_ih, b_hh)
    onehot3 = _prep_onehot3()
    in_maps = []
    for core in range(NCORES):
        in_maps.append(
            {
                "tokens": _prep_tokens(
                    leaf_tokens[core * LEAVES_CORE : (core + 1) * LEAVES_CORE]
                ),
                "embed": embed_bf,
                "wih_t": wih_t,
                "wih_s": wih_s,
                "whh_t": whh_t,
                "biases": biases,
                "biases_mm": biases_mm,
                "onehot3": onehot3,
            }
        )
    res = run_bass_kernel_spmd(nc, in_maps, core_ids=list(range(NCORES)))

    # device tensors -> [core, 64 nodes, 384] with f = j*128 + p
    x3 = np.zeros((NCORES, P4, DIM), np.float64)
    h3 = np.zeros((NCORES, P4, DIM), np.float64)
    for core in range(NCORES):
        hacc = np.asarray(res.results[core]["out_hacc"], np.float64)  # [128,3,64]
        hf = np.asarray(res.results[core]["out_hf"], np.float64)
        x3[core] = (hacc / ARITY).transpose(1, 0, 2).reshape(DIM, P4).T
        h3[core] = hf.transpose(1, 0, 2).reshape(DIM, P4).T

    w_ih64 = w_ih.astype(np.float64)
    w_hh64 = w_hh.astype(np.float64)
    b_ih64 = b_ih.astype(np.float64)
    b_hh64 = b_hh.astype(np.float64)

    # level 3: per core, 8 parents x 8 children (batch over cores*parents)
    xc = x3.reshape(NCORES * ARITY, ARITY, DIM).transpose(1, 0, 2)  # [A, 64, D]
    h0 = h3.reshape(NCORES * ARITY, ARITY, DIM).mean(axis=1)
    x2, h2 = _gru_level(xc, h0, w_ih64, w_hh64, b_ih64, b_hh64)

    # level 2: per core, 1 parent x 8 children
    xc = x2.reshape(NCORES, ARITY, DIM).transpose(1, 0, 2)  # [A, 8, D]
    h0 = h2.reshape(NCORES, ARITY, DIM).mean(axis=1)
    x1, h1 = _gru_level(xc, h0, w_ih64, w_hh64, b_ih64, b_hh64)

    # root: 8 cores' outputs
    xc = x1.reshape(1, ARITY, DIM).transpose(1, 0, 2)  # [A, 1, D]
    h0 = h1.reshape(1, ARITY, DIM).mean(axis=1)
    out, _ = _gru_level(xc, h0, w_ih64, w_hh64, b_ih64, b_hh64)

    return out.astype(np.float32).reshape(1, 1, DIM)


# revision 15
# speedup vs baseline: 1.2981x; 1.2981x over previous
"""Tree-GRU (arity-8, depth-5) over embedded leaves on 8 TRN2 NeuronCores.

Sharding: data-parallel over subtrees. Each core takes 4096 contiguous leaves
and runs levels 5 and 4 of the tree locally (512 -> 64 parents). The last two
per-core levels (64 -> 8 -> 1) and the root are small latency-bound GRU
cascades (free dim <= 8) done on host in fp64 after gathering the per-core
level-4 outputs, extending the baseline's host-side root reduction.

Device layout is feature-transposed: tensors live as [128 part, 3 ktile, ...]
with feature f = 128*k + p, so the GRU matmuls contract the partition dim.

Embeddings arrive per GRU step: tokens are host-permuted child-major, each
child's 512 rows fetched by 4 indirect DMAs (leaf-major) and flipped
feature-major by 4 xbar transpose-DMAs on the HWDGE rings — no tensor-engine
transposes, no PSUM, and the first GRU matmul can start after ~2 gathers.

Level 512 keeps one PSUM bank per (role, jo) output tile at N=512; each step
emits gi matmuls of units j0/j1 ahead of any hh matmul so the tensor engine
holds ~3.8us of h-independent work to hide the previous step's gate chain.
Unit j1 owns 4 banks (double-buffered step to step); j0 and j2 share the
other 4, with j2's allocation waiting on j0's progressively-freed banks
behind hh j1. Biases ride the scalar-activation bias port. The per-step
output accumulator is kept child-major (vector engine) so the final step
writes level 4's input directly as a fused raw-sum add; the 1/8 output-mean
scale is folded into a pre-scaled copy of W_ih used by level 4. Level 64
injects biases into PSUM via a K=3 one-hot matmul (the only start=True
write), collapsing the gate chain to jo-spanning instructions.
"""

import numpy as np
import ml_dtypes

ARITY = 8
DIM = 384
VOCAB = 32000
NCORES = 8
P = 128
J = 3  # DIM // 128 feature tiles
N_LEAVES = 32768
LEAVES_CORE = N_LEAVES // NCORES  # 4096
P5 = LEAVES_CORE // ARITY  # 512 level-5 parents per core
P4 = P5 // ARITY  # 64 level-4 parents per core
GT = P5 // P  # 4 gather tiles per child

BF16 = ml_dtypes.bfloat16

_PROG_CACHE = {}


def _emit(tc, nc, aps):
    import concourse.mybir as mybir
    import concourse.bass as bass

    f32 = mybir.dt.float32
    bf16 = mybir.dt.bfloat16
    Sig = mybir.ActivationFunctionType.Sigmoid
    Tanh = mybir.ActivationFunctionType.Tanh
    Add = mybir.AluOpType.add
    Sub = mybir.AluOpType.subtract
    Mult = mybir.AluOpType.mult

    tokens, embed, wih_t, wih_s, whh_t, biases, biases_mm, onehot3, out_hacc, out_hf = aps

    from contextlib import ExitStack

    with ExitStack() as ctx:
        const = ctx.enter_context(tc.tile_pool(name="const", bufs=1))
        xpool = ctx.enter_context(tc.tile_pool(name="xpool", bufs=1))
        gpool = ctx.enter_context(tc.tile_pool(name="gpool", bufs=3))
        state = ctx.enter_context(tc.tile_pool(name="state", bufs=1))
        gates = ctx.enter_context(tc.tile_pool(name="gates", bufs=4))
        pspool = ctx.enter_context(tc.tile_pool(name="pspool", bufs=4, space="PSUM"))
        pspool2 = ctx.enter_context(tc.tile_pool(name="pspool2", bufs=4, space="PSUM"))

        # ---- all constant loads on the sync ring, in order, ahead of the
        # xbar transposes (one FIFO: no cross-ring semaphore serialization) ----
        tok_sb = const.tile([P, ARITY * GT], mybir.dt.int32)
        nc.sync.dma_start(tok_sb[:], tokens[:])

        wih_sb = const.tile([P, J, 9, P], bf16)
        wih_s_sb = const.tile([P, J, 9, P], bf16)
        whh_sb = const.tile([P, J, 9, P], bf16)
        bias_sb = const.tile([P, 12], f32)
        bias3_sb = const.tile([3, 4, P], bf16)
        onehot3_sb = const.tile([3, 3, 512], bf16)
        nc.sync.dma_start(wih_sb[:], wih_t[:])
        nc.sync.dma_start(whh_sb[:], whh_t[:])
        nc.sync.dma_start(wih_s_sb[:], wih_s[:])
        nc.sync.dma_start(bias_sb[:], biases[:])
        nc.sync.dma_start(bias3_sb[:], biases_mm[:])
        nc.sync.dma_start(onehot3_sb[:], onehot3[:])

        # ---- per-child gather (leaf-major) + xbar transpose to feature-major ----
        x5 = xpool.tile([P, ARITY, J, P5], bf16, name="x5", tag="x5")
        for t in range(ARITY):
            c = ARITY - 1 - t  # children consumed in reverse: child 7 first
            xg = gpool.tile([P, GT, DIM], bf16, name="xg", tag="xg")
            for g in range(GT):
                gi_inst = nc.gpsimd.indirect_dma_start(
                    out=xg[:, g, :],
                    out_offset=None,
                    in_=embed[:],
                    in_offset=bass.IndirectOffsetOnAxis(
                        ap=tok_sb[:, c * GT + g : c * GT + g + 1], axis=0
                    ),
                )
                if g % 2 == 1:
                    gi_inst.ins.queue = "qPoolDynamic1"
            for g in range(GT):
                nc.sync.dma_start_transpose(
                    x5[:, c, :, g * P : (g + 1) * P], xg[:, g, :]
                )

        x4 = xpool.tile([P, ARITY, J, P4], bf16, name="x4", tag="x4")

        def psum_tile(jo):
            # 8 banks for 12 role-tiles per step: unit j1 owns pspool (reuse
            # waits on the previous step's j1 gates); j0/j2 share pspool2 —
            # j2 waits on same-step j0 gates (freed progressively under hh
            # j1), j0 on the previous step's j2 gates. All waits point at
            # strictly earlier FIFO positions: no deadlock.
            if jo == 1:
                return pspool.tile([P, 512], f32, name="ps", tag="ps")
            return pspool2.tile([P, 512], f32, name="ps2", tag="ps2")

        # =================== level 5: 512 parents, leaf children ===================
        h5 = state.tile([P, J, P5], bf16, name="h5", tag="h5")
        hacc5 = state.tile([P, J, ARITY, P4], f32, name="hacc5", tag="hacc5")
        nc.gpsimd.memset(hacc5[:], 0.0)
        csum5 = state.tile([P, J, P4], f32, name="csum5", tag="csum5")

        with nc.named_scope("level_512"):
            for t in range(ARITY):
                c = ARITY - 1 - t
                leaf0 = t == 0

                ps_r = [None] * J
                ps_z = [None] * J
                ps_in = [None] * J
                ps_hn = [None] * J

                def emit_gi(jo):
                    ps_r[jo] = psum_tile(jo)
                    ps_z[jo] = psum_tile(jo)
                    ps_in[jo] = psum_tile(jo)
                    if not leaf0:
                        ps_hn[jo] = psum_tile(jo)
                    for ps, moff in ((ps_r[jo], 0), (ps_z[jo], 3), (ps_in[jo], 6)):
                        for ji in range(J):
                            nc.tensor.matmul(
                                ps[:, :P5],
                                wih_sb[:, ji, moff + jo, :],
                                x5[:, c, ji, :],
                                start=(ji == 0),
                                stop=(ji == 2 and (moff == 6 or leaf0)),
                            )

                def emit_hh(jo):
                    if leaf0:
                        return
                    for ps, moff in ((ps_r[jo], 0), (ps_z[jo], 3), (ps_hn[jo], 6)):
                        for ji in range(J):
                            nc.tensor.matmul(
                                ps[:, :P5],
                                whh_sb[:, ji, moff + jo, :],
                                h5[:, ji, :],
                                start=(ji == 0 and moff == 6),
                                stop=(ji == 2),
                            )

                emit_gi(0)
                emit_gi(1)
                emit_hh(0)
                emit_hh(1)
                emit_gi(2)
                emit_hh(2)

                for jo in range(J):
                    r_sb = gates.tile([P, P5], bf16, name="r_sb", tag="r_sb")
                    z_sb = gates.tile([P, P5], bf16, name="z_sb", tag="z_sb")
                    n_sb = gates.tile([P, P5], bf16, name="n_sb", tag="n_sb")
                    rhn = gates.tile([P, P5], f32, name="rhn", tag="rhn")
                    t1 = gates.tile([P, P5], bf16, name="t1", tag="t1")

                    nc.scalar.activation(
                        r_sb[:], ps_r[jo][:, :P5], Sig, bias=bias_sb[:, jo : jo + 1]
                    )
                    nc.scalar.activation(
                        z_sb[:], ps_z[jo][:, :P5], Sig, bias=bias_sb[:, 3 + jo : 4 + jo]
                    )
                    if leaf0:
                        nc.vector.tensor_scalar_mul(
                            rhn[:], r_sb[:], bias_sb[:, 6 + jo : 7 + jo]
                        )
                    else:
                        nc.vector.scalar_tensor_tensor(
                            out=rhn[:],
                            in0=ps_hn[jo][:, :P5],
                            scalar=bias_sb[:, 6 + jo : 7 + jo],
                            in1=r_sb[:],
                            op0=Add,
                            op1=Mult,
                        )
                    nc.vector.tensor_tensor(
                        out=rhn[:], in0=rhn[:], in1=ps_in[jo][:, :P5], op=Add
                    )
                    nc.scalar.activation(
                        n_sb[:], rhn[:], Tanh, bias=bias_sb[:, 9 + jo : 10 + jo]
                    )
                    hsl = h5[:, jo, :]
                    if leaf0:
                        nc.vector.tensor_tensor(out=t1[:], in0=z_sb[:], in1=n_sb[:], op=Mult)
                        nc.vector.tensor_tensor(out=hsl, in0=n_sb[:], in1=t1[:], op=Sub)
                    else:
                        nc.vector.tensor_tensor(out=t1[:], in0=hsl, in1=n_sb[:], op=Sub)
                        nc.vector.tensor_tensor(out=t1[:], in0=z_sb[:], in1=t1[:], op=Mult)
                        nc.vector.tensor_tensor(out=hsl, in0=n_sb[:], in1=t1[:], op=Add)

                # output accumulation after the chain ops so it never delays
                # the next step's recurrent matmuls
                for jo in range(J):
                    hsl = h5[:, jo, :]
                    hperm = hsl.rearrange("p (q c) -> p c q", c=ARITY)
                    if t == ARITY - 1:
                        nc.vector.tensor_reduce(
                            out=csum5[:, jo, :],
                            in_=hsl.rearrange("p (q c) -> p q c", c=ARITY),
                            axis=mybir.AxisListType.X,
                            op=Add,
                        )
                        nc.vector.tensor_tensor(
                            out=x4[:, :, jo, :],
                            in0=hacc5[:, jo],
                            in1=hperm,
                            op=Add,
                        )
                    else:
                        nc.vector.tensor_tensor(
                            out=hacc5[:, jo], in0=hacc5[:, jo], in1=hperm, op=Add
                        )

        # =================== level 4: 64 parents ===================
        h4 = state.tile([P, J, P4], bf16, name="h4", tag="h4")
        nc.scalar.mul(h4[:], csum5[:], 1.0 / ARITY)
        hacc4 = state.tile([P, J, P4], f32, name="hacc4", tag="hacc4")
        nc.gpsimd.memset(hacc4[:], 0.0)
        N3 = J * P4  # 192

        with nc.named_scope("level_64"):
            for t in range(ARITY):
                c = ARITY - 1 - t
                pool = pspool if t % 2 == 0 else pspool2
                tag = "ps" if t % 2 == 0 else "ps2"
                ps_r, ps_z, ps_hn, ps_in = (
                    pool.tile([P, 512], f32, name=tag, tag=tag) for _ in range(4)
                )

                def view3(pst):
                    return pst[:, :N3].rearrange("p (j n) -> p j n", j=3)

                # bias matmul: the only start=True write, covering each tile
                for pst, ro in ((ps_r, 0), (ps_z, 1), (ps_hn, 2), (ps_in, 3)):
                    nc.tensor.matmul(
                        pst[:, :N3],
                        bias3_sb[:, ro, :],
                        onehot3_sb[:, :, :P4],
                        start=True,
                        stop=False,
                    )
                for pst, moff in ((ps_r, 0), (ps_z, 3), (ps_in, 6)):
                    for jo in range(J):
                        for ji in range(J):
                            nc.tensor.matmul(
                                pst[:, jo * P4 : (jo + 1) * P4],
                                wih_s_sb[:, ji, moff + jo, :],
                                x4[:, c, ji, :],
                                start=False,
                                stop=(moff == 6 and jo == 2 and ji == 2),
                            )
                for pst, moff in ((ps_r, 0), (ps_z, 3), (ps_hn, 6)):
                    for jo in range(J):
                        for ji in range(J):
                            nc.tensor.matmul(
                                pst[:, jo * P4 : (jo + 1) * P4],
                                whh_sb[:, ji, moff + jo, :],
                                h4[:, ji, :],
                                start=False,
                                stop=(jo == 2 and ji == 2),
                            )

                r_sb = gates.tile([P, J, P4], bf16, name="r4", tag="r4")
                z_sb = gates.tile([P, J, P4], bf16, name="z4", tag="z4")
                n_sb = gates.tile([P, J, P4], bf16, name="n4", tag="n4")
                rhn = gates.tile([P, J, P4], f32, name="rhn4", tag="rhn4")
                t1 = gates.tile([P, J, P4], bf16, name="t14", tag="t14")

                nc.scalar.activation(r_sb[:], view3(ps_r), Sig)
                nc.scalar.activation(z_sb[:], view3(ps_z), Sig)
                nc.vector.tensor_tensor(
                    out=rhn[:], in0=view3(ps_hn), in1=r_sb[:], op=Mult
                )
                nc.vector.tensor_tensor(
                    out=rhn[:], in0=rhn[:], in1=view3(ps_in), op=Add
                )
                nc.scalar.activation(n_sb[:], rhn[:], Tanh)
                nc.vector.tensor_tensor(out=t1[:], in0=h4[:], in1=n_sb[:], op=Sub)
                nc.vector.tensor_tensor(out=t1[:], in0=z_sb[:], in1=t1[:], op=Mult)
                nc.vector.tensor_tensor(out=h4[:], in0=n_sb[:], in1=t1[:], op=Add)
                nc.vector.tensor_tensor(
                    out=hacc4[:], in0=hacc4[:], in1=h4[:], op=Add
                )

        # ---- outputs: raw h-sum (x3*8) and final hiddens of the 64 nodes ----
        nc.sync.dma_start(out_hacc[:], hacc4[:])
        nc.sync.dma_start(out_hf[:], h4[:])


def _build_program():
    if "prog" in _PROG_CACHE:
        return _PROG_CACHE["prog"]
    import concourse.bacc as bacc
    import concourse.mybir as mybir
    import concourse.tile as tile

    f32 = mybir.dt.float32
    bf16 = mybir.dt.bfloat16

    nc = bacc.Bacc(
        "TRN2",
        target_bir_lowering=False,
        debug=False,
        enable_asserts=False,
        num_devices=NCORES,
        num_swdge_queues=2,
    )
    tokens = nc.dram_tensor(
        "tokens", [P, ARITY * GT], mybir.dt.int32, kind="ExternalInput"
    ).ap()
    embed = nc.dram_tensor("embed", [VOCAB, DIM], bf16, kind="ExternalInput").ap()
    wih_t = nc.dram_tensor("wih_t", [P, J, 9, P], bf16, kind="ExternalInput").ap()
    wih_s = nc.dram_tensor("wih_s", [P, J, 9, P], bf16, kind="ExternalInput").ap()
    whh_t = nc.dram_tensor("whh_t", [P, J, 9, P], bf16, kind="ExternalInput").ap()
    biases = nc.dram_tensor("biases", [P, 12], f32, kind="ExternalInput").ap()
    biases_mm = nc.dram_tensor("biases_mm", [3, 4, P], bf16, kind="ExternalInput").ap()
    onehot3 = nc.dram_tensor("onehot3", [3, 3, 512], bf16, kind="ExternalInput").ap()
    out_hacc = nc.dram_tensor("out_hacc", [P, J, P4], f32, kind="ExternalOutput").ap()
    out_hf = nc.dram_tensor("out_hf", [P, J, P4], bf16, kind="ExternalOutput").ap()

    with tile.TileContext(nc) as tc:
        _emit(
            tc,
            nc,
            (
                tokens,
                embed,
                wih_t,
                wih_s,
                whh_t,
                biases,
                biases_mm,
                onehot3,
                out_hacc,
                out_hf,
            ),
        )
    nc.compile()
    _PROG_CACHE["prog"] = nc
    return nc


def _retile_weights(w):
    # w: [1152, 384] -> lhsT tiles [128(k_part), 3(k), 9(m), 128(m_col)] bf16
    wt = np.ascontiguousarray(w.T)  # [384, 1152]
    wt = wt.reshape(J, P, 9, P).transpose(1, 0, 2, 3)
    return np.ascontiguousarray(wt).astype(BF16)


def _prep_bias(b_ih, b_hh):
    biases = np.zeros((P, 12), np.float32)
    comb = (b_ih + b_hh).reshape(9, P)
    biases[:, 0:6] = comb[0:6].T
    biases[:, 6:9] = b_hh.reshape(9, P)[6:9].T
    biases[:, 9:12] = b_ih.reshape(9, P)[6:9].T
    return biases


def _prep_bias_mm(b_ih, b_hh):
    # lhsT[k, ro, q] = bias[q, 3*ro + k]: the K=3 bias matmul against the
    # one-hot rhs yields out[q, (j, n)] = bias[q, 3*ro + j].
    b = _prep_bias(b_ih, b_hh)  # [128, 12] cols: r0..2 z0..2 hn0..2 in0..2
    out = b.T.reshape(4, 3, P).transpose(1, 0, 2)
    return np.ascontiguousarray(out).astype(BF16)


def _prep_onehot3():
    out = np.zeros((3, 3, 512), np.float32)
    for k in range(3):
        out[k, k, :] = 1.0
    return out.astype(BF16)


def _prep_tokens(tokens_core):
    # [128, 8*4] with col c*4+g holding tokens[(g*128 + p)*8 + c]: contiguous
    # per-partition rows so the token DMA is a handful of descriptors.
    tok = tokens_core.reshape(P5, ARITY).T  # [8 child, 512 parent]
    return np.ascontiguousarray(tok.reshape(ARITY, GT, P).transpose(2, 0, 1).reshape(P, ARITY * GT))


def _gru_level(x_children, h0, w_ih, w_hh, b_ih, b_hh):
    # x_children: [A, N, D] in original child order; consumed reversed.
    h = h0
    acc = np.zeros_like(h)
    for t in range(ARITY):
        x_t = x_children[ARITY - 1 - t]
        gi = x_t @ w_ih.T + b_ih
        gh = h @ w_hh.T + b_hh
        i_r, i_z, i_n = np.split(gi, 3, axis=-1)
        h_r, h_z, h_n = np.split(gh, 3, axis=-1)
        r = 1.0 / (1.0 + np.exp(-(i_r + h_r)))
        z = 1.0 / (1.0 + np.exp(-(i_z + h_z)))
        n = np.tanh(i_n + r * h_n)
        h = (1.0 - z) * n + z * h
        acc += h
    return acc / ARITY, h


def kernel(leaf_tokens, embed_table, w_ih, w_hh, b_ih, b_hh):
    from concourse.bass_utils import run_bass_kernel_spmd

    leaf_tokens = np.asarray(leaf_tokens, np.int32)
    embed_table = np.asarray(embed_table, np.float32)
    w_ih = np.asarray(w_ih, np.float32)
    w_hh = np.asarray(w_hh, np.float32)
    b_ih = np.asarray(b_ih, np.float32)
    b_hh = np.asarray(b_hh, np.float32)

    nc = _build_program()

    embed_bf = embed_table.astype(BF16)
    wih_t = _retile_weights(w_ih)
    wih_s = _retile_weights(w_ih / ARITY)
    whh_t = _retile_weights(w_hh)
    biases = _prep_bias(b_ih, b_hh)
    biases_mm = _prep_bias_mm(b_ih, b_hh)
    onehot3 = _prep_onehot3()
    in_maps = []
    for core in range(NCORES):
        in_maps.append(
            {
                "tokens": _prep_tokens(
                    leaf_tokens[core * LEAVES_CORE : (core + 1) * LEAVES_CORE]
                ),
                "embed": embed_bf,
                "wih_t": wih_t,
                "wih_s": wih_s,
                "whh_t": whh_t,
                "biases": biases,
                "biases_mm": biases_mm,
                "onehot3": onehot3,
            }
        )
    res = run_bass_kernel_spmd(nc, in_maps, core_ids=list(range(NCORES)))

    # device tensors -> [core, 64 nodes, 384] with f = j*128 + p
    x3 = np.zeros((NCORES, P4, DIM), np.float64)
    h3 = np.zeros((NCORES, P4, DIM), np.float64)
    for core in range(NCORES):
        hacc = np.asarray(res.results[core]["out_hacc"], np.float64)  # [128,3,64]
        hf = np.asarray(res.results[core]["out_hf"], np.float64)
        x3[core] = (hacc / ARITY).transpose(1, 0, 2).reshape(DIM, P4).T
        h3[core] = hf.transpose(1, 0, 2).reshape(DIM, P4).T

    w_ih64 = w_ih.astype(np.float64)
    w_hh64 = w_hh.astype(np.float64)
    b_ih64 = b_ih.astype(np.float64)
    b_hh64 = b_hh.astype(np.float64)

    # level 3: per core, 8 parents x 8 children (batch over cores*parents)
    xc = x3.reshape(NCORES * ARITY, ARITY, DIM).transpose(1, 0, 2)  # [A, 64, D]
    h0 = h3.reshape(NCORES * ARITY, ARITY, DIM).mean(axis=1)
    x2, h2 = _gru_level(xc, h0, w_ih64, w_hh64, b_ih64, b_hh64)

    # level 2: per core, 1 parent x 8 children
    xc = x2.reshape(NCORES, ARITY, DIM).transpose(1, 0, 2)  # [A, 8, D]
    h0 = h2.reshape(NCORES, ARITY, DIM).mean(axis=1)
    x1, h1 = _gru_level(xc, h0, w_ih64, w_hh64, b_ih64, b_hh64)

    # root: 8 cores' outputs
    xc = x1.reshape(1, ARITY, DIM).transpose(1, 0, 2)  # [A, 1, D]
    h0 = h1.reshape(1, ARITY, DIM).mean(axis=1)
    out, _ = _gru_level(xc, h0, w_ih64, w_hh64, b_ih64, b_hh64)

    return out.astype(np.float32).reshape(1, 1, DIM)


# revision 19
# speedup vs baseline: 1.4275x; 1.0996x over previous
"""Tree-GRU (arity-8, depth-5) over embedded leaves on 8 TRN2 NeuronCores.

Sharding: data-parallel over subtrees. Each core takes 4096 contiguous leaves
and runs levels 5 and 4 of the tree locally (512 -> 64 parents). The last two
per-core levels (64 -> 8 -> 1) and the root are small latency-bound GRU
cascades (free dim <= 8) done on host in fp64 after gathering the per-core
level-4 outputs, extending the baseline's host-side root reduction.

Device layout is feature-transposed: tensors live as [128 part, 3 ktile, ...]
with feature f = 128*k + p, so the GRU matmuls contract the partition dim.

Embeddings arrive per GRU step: tokens are host-permuted child-major, each
child's 512 rows fetched by 4 indirect DMAs (leaf-major) and flipped
feature-major by 4 xbar transpose-DMAs on the HWDGE rings — no tensor-engine
transposes, no PSUM, and the first GRU matmul can start after ~2 gathers.

Level 512 keeps one PSUM bank per (role, jo) output tile at N=512; each step
emits gi matmuls of units j0/j1 ahead of any hh matmul so the tensor engine
holds ~3.8us of h-independent work to hide the previous step's gate chain.
Unit j1 owns 4 banks (double-buffered step to step); j0 and j2 share the
other 4, with j2's allocation waiting on j0's progressively-freed banks
behind hh j1. Biases ride the scalar-activation bias port. The per-step
output accumulator is kept child-major (vector engine) so the final step
writes level 4's input directly as a fused raw-sum add; the 1/8 output-mean
scale is folded into a pre-scaled copy of W_ih used by level 4. Level 64
injects biases into PSUM via a K=3 one-hot matmul (the only start=True
write), collapsing the gate chain to jo-spanning instructions.
"""

import numpy as np
import ml_dtypes

ARITY = 8
DIM = 384
VOCAB = 32000
NCORES = 8
P = 128
J = 3  # DIM // 128 feature tiles
N_LEAVES = 32768
LEAVES_CORE = N_LEAVES // NCORES  # 4096
P5 = LEAVES_CORE // ARITY  # 512 level-5 parents per core
P4 = P5 // ARITY  # 64 level-4 parents per core
GT = P5 // P  # 4 gather tiles per child

BF16 = ml_dtypes.bfloat16

_PROG_CACHE = {}


def _emit(tc, nc, aps):
    import concourse.mybir as mybir
    import concourse.bass as bass

    f32 = mybir.dt.float32
    bf16 = mybir.dt.bfloat16
    Sig = mybir.ActivationFunctionType.Sigmoid
    Tanh = mybir.ActivationFunctionType.Tanh
    Add = mybir.AluOpType.add
    Sub = mybir.AluOpType.subtract
    Mult = mybir.AluOpType.mult

    tokens, embed, wih_t, wih_s, whh_t, biases, biases_mm, onehot3, out_hacc, out_hf = aps

    from contextlib import ExitStack

    with ExitStack() as ctx:
        const = ctx.enter_context(tc.tile_pool(name="const", bufs=1))
        xpool = ctx.enter_context(tc.tile_pool(name="xpool", bufs=1))
        gpool = ctx.enter_context(tc.tile_pool(name="gpool", bufs=3))
        state = ctx.enter_context(tc.tile_pool(name="state", bufs=1))
        gates = ctx.enter_context(tc.tile_pool(name="gates", bufs=4))
        pspool = ctx.enter_context(tc.tile_pool(name="pspool", bufs=4, space="PSUM"))
        pspool2 = ctx.enter_context(tc.tile_pool(name="pspool2", bufs=4, space="PSUM"))

        # ---- index tile first, then feature-major embedding gathers ----
        HALF = P5 // 2  # 256 indices per dma_gather
        idx_sb = const.tile([P, ARITY * 2 * (HALF // 16)], mybir.dt.int16)
        nc.sync.dma_start(idx_sb[:], tokens[:])

        wih_sb = const.tile([P, J, 9, P], bf16)
        wih_s_sb = const.tile([P, J, 9, P], bf16)
        whh_sb = const.tile([P, J, 9, P], bf16)
        bias_sb = const.tile([P, 12], f32)
        bias3_sb = const.tile([3, 4, P], bf16)
        onehot3_sb = const.tile([3, 3, 512], bf16)
        nc.sync.dma_start(wih_sb[:], wih_t[:])
        nc.sync.dma_start(whh_sb[:], whh_t[:])
        nc.sync.dma_start(wih_s_sb[:], wih_s[:])
        nc.sync.dma_start(bias_sb[:], biases[:])
        nc.sync.dma_start(bias3_sb[:], biases_mm[:])
        nc.sync.dma_start(onehot3_sb[:], onehot3[:])

        # dummy gather off a zeroed index tile: its only job is to be the
        # first mlp-library instruction so the ~13us Q7 library load runs
        # during the index DMA instead of serializing before child 7
        idx0 = const.tile([P, 8], mybir.dt.int16)
        nc.gpsimd.memset(idx0[:], 0)
        warm = gpool.tile([P, J, P], bf16, name="warm", tag="warm")
        nc.gpsimd.dma_gather(warm[:], embed[:], idx0[:], P, P, DIM, transpose=True)

        # x5[p, child, half, j, q] with q in [0, 256): each (child, half)
        # gather lands contiguously; matmul rhs spans both halves as a 2D
        # free AP [2, 256] = 512 columns
        x5 = xpool.tile([P, ARITY, 2, J, HALF], bf16, name="x5", tag="x5")
        ncols = HALF // 16
        for t in range(ARITY):
            c = ARITY - 1 - t  # children consumed in reverse: child 7 first
            for half in range(2):
                blk = c * 2 + half
                nc.gpsimd.dma_gather(
                    x5[:, c, half],
                    embed[:],
                    idx_sb[:, blk * ncols : (blk + 1) * ncols],
                    HALF,
                    HALF,
                    DIM,
                    transpose=True,
                    queue_num=half,
                )

        x4 = xpool.tile([P, ARITY, J, P4], bf16, name="x4", tag="x4")

        def psum_tile(jo):
            # 8 banks for 12 role-tiles per step: unit j1 owns pspool (reuse
            # waits on the previous step's j1 gates); j0/j2 share pspool2 —
            # j2 waits on same-step j0 gates (freed progressively under hh
            # j1), j0 on the previous step's j2 gates. All waits point at
            # strictly earlier FIFO positions: no deadlock.
            if jo == 1:
                return pspool.tile([P, 512], f32, name="ps", tag="ps")
            return pspool2.tile([P, 512], f32, name="ps2", tag="ps2")

        # =================== level 5: 512 parents, leaf children ===================
        h5 = state.tile([P, J, P5], bf16, name="h5", tag="h5")
        hacc5 = state.tile([P, J, ARITY, P4], f32, name="hacc5", tag="hacc5")
        nc.gpsimd.memset(hacc5[:], 0.0)
        csum5 = state.tile([P, J, P4], f32, name="csum5", tag="csum5")

        with nc.named_scope("level_512"):
            for t in range(ARITY):
                c = ARITY - 1 - t
                leaf0 = t == 0

                ps_r = [None] * J
                ps_z = [None] * J
                ps_in = [None] * J
                ps_hn = [None] * J

                def emit_gi(jo):
                    ps_r[jo] = psum_tile(jo)
                    ps_z[jo] = psum_tile(jo)
                    ps_in[jo] = psum_tile(jo)
                    if not leaf0:
                        ps_hn[jo] = psum_tile(jo)
                    for ps, moff in ((ps_r[jo], 0), (ps_z[jo], 3), (ps_in[jo], 6)):
                        for ji in range(J):
                            nc.tensor.matmul(
                                ps[:, :P5],
                                wih_sb[:, ji, moff + jo, :],
                                x5[:, c, :, ji, :],
                                start=(ji == 0),
                                stop=(ji == 2 and (moff == 6 or leaf0)),
                            )

                def emit_hh(jo):
                    if leaf0:
                        return
                    for ps, moff in ((ps_r[jo], 0), (ps_z[jo], 3), (ps_hn[jo], 6)):
                        for ji in range(J):
                            nc.tensor.matmul(
                                ps[:, :P5],
                                whh_sb[:, ji, moff + jo, :],
                                h5[:, ji, :],
                                start=(ji == 0 and moff == 6),
                                stop=(ji == 2),
                            )

                emit_gi(0)
                emit_gi(1)
                emit_hh(0)
                emit_hh(1)
                emit_gi(2)
                emit_hh(2)

                for jo in range(J):
                    r_sb = gates.tile([P, P5], bf16, name="r_sb", tag="r_sb")
                    z_sb = gates.tile([P, P5], bf16, name="z_sb", tag="z_sb")
                    n_sb = gates.tile([P, P5], bf16, name="n_sb", tag="n_sb")
                    rhn = gates.tile([P, P5], f32, name="rhn", tag="rhn")
                    t1 = gates.tile([P, P5], bf16, name="t1", tag="t1")

                    nc.scalar.activation(
                        r_sb[:], ps_r[jo][:, :P5], Sig, bias=bias_sb[:, jo : jo + 1]
                    )
                    nc.scalar.activation(
                        z_sb[:], ps_z[jo][:, :P5], Sig, bias=bias_sb[:, 3 + jo : 4 + jo]
                    )
                    if leaf0:
                        nc.vector.tensor_scalar_mul(
                            rhn[:], r_sb[:], bias_sb[:, 6 + jo : 7 + jo]
                        )
                    else:
                        nc.vector.scalar_tensor_tensor(
                            out=rhn[:],
                            in0=ps_hn[jo][:, :P5],
                            scalar=bias_sb[:, 6 + jo : 7 + jo],
                            in1=r_sb[:],
                            op0=Add,
                            op1=Mult,
                        )
                    nc.vector.tensor_tensor(
                        out=rhn[:], in0=rhn[:], in1=ps_in[jo][:, :P5], op=Add
                    )
                    nc.scalar.activation(
                        n_sb[:], rhn[:], Tanh, bias=bias_sb[:, 9 + jo : 10 + jo]
                    )
                    hsl = h5[:, jo, :]
                    if leaf0:
                        nc.vector.tensor_tensor(out=t1[:], in0=z_sb[:], in1=n_sb[:], op=Mult)
                        nc.vector.tensor_tensor(out=hsl, in0=n_sb[:], in1=t1[:], op=Sub)
                    else:
                        nc.vector.tensor_tensor(out=t1[:], in0=hsl, in1=n_sb[:], op=Sub)
                        nc.vector.tensor_tensor(out=t1[:], in0=z_sb[:], in1=t1[:], op=Mult)
                        nc.vector.tensor_tensor(out=hsl, in0=n_sb[:], in1=t1[:], op=Add)

                # output accumulation after the chain ops so it never delays
                # the next step's recurrent matmuls
                for jo in range(J):
                    hsl = h5[:, jo, :]
                    hperm = hsl.rearrange("p (q c) -> p c q", c=ARITY)
                    if t == ARITY - 1:
                        nc.vector.tensor_reduce(
                            out=csum5[:, jo, :],
                            in_=hsl.rearrange("p (q c) -> p q c", c=ARITY),
                            axis=mybir.AxisListType.X,
                            op=Add,
                        )
                        nc.vector.tensor_tensor(
                            out=x4[:, :, jo, :],
                            in0=hacc5[:, jo],
                            in1=hperm,
                            op=Add,
                        )
                    else:
                        nc.vector.tensor_tensor(
                            out=hacc5[:, jo], in0=hacc5[:, jo], in1=hperm, op=Add
                        )

        # =================== level 4: 64 parents ===================
        h4 = state.tile([P, J, P4], bf16, name="h4", tag="h4")
        nc.scalar.mul(h4[:], csum5[:], 1.0 / ARITY)
        hacc4 = state.tile([P, J, P4], f32, name="hacc4", tag="hacc4")
        nc.gpsimd.memset(hacc4[:], 0.0)
        N3 = J * P4  # 192

        with nc.named_scope("level_64"):
            for t in range(ARITY):
                c = ARITY - 1 - t
                pool = pspool if t % 2 == 0 else pspool2
                tag = "ps" if t % 2 == 0 else "ps2"
                ps_r, ps_z, ps_hn, ps_in = (
                    pool.tile([P, 512], f32, name=tag, tag=tag) for _ in range(4)
                )

                def view3(pst):
                    return pst[:, :N3].rearrange("p (j n) -> p j n", j=3)

                # bias matmul: the only start=True write, covering each tile
                for pst, ro in ((ps_r, 0), (ps_z, 1), (ps_hn, 2), (ps_in, 3)):
                    nc.tensor.matmul(
                        pst[:, :N3],
                        bias3_sb[:, ro, :],
                        onehot3_sb[:, :, :P4],
                        start=True,
                        stop=False,
                    )
                for pst, moff in ((ps_r, 0), (ps_z, 3), (ps_in, 6)):
                    for jo in range(J):
                        for ji in range(J):
                            nc.tensor.matmul(
                                pst[:, jo * P4 : (jo + 1) * P4],
                                wih_s_sb[:, ji, moff + jo, :],
                                x4[:, c, ji, :],
                                start=False,
                                stop=(moff == 6 and jo == 2 and ji == 2),
                            )
                for pst, moff in ((ps_r, 0), (ps_z, 3), (ps_hn, 6)):
                    for jo in range(J):
                        for ji in range(J):
                            nc.tensor.matmul(
                                pst[:, jo * P4 : (jo + 1) * P4],
                                whh_sb[:, ji, moff + jo, :],
                                h4[:, ji, :],
                                start=False,
                                stop=(jo == 2 and ji == 2),
                            )

                r_sb = gates.tile([P, J, P4], bf16, name="r4", tag="r4")
                z_sb = gates.tile([P, J, P4], bf16, name="z4", tag="z4")
                n_sb = gates.tile([P, J, P4], bf16, name="n4", tag="n4")
                rhn = gates.tile([P, J, P4], f32, name="rhn4", tag="rhn4")
                t1 = gates.tile([P, J, P4], bf16, name="t14", tag="t14")

                nc.scalar.activation(r_sb[:], view3(ps_r), Sig)
                nc.scalar.activation(z_sb[:], view3(ps_z), Sig)
                nc.vector.tensor_tensor(
                    out=rhn[:], in0=view3(ps_hn), in1=r_sb[:], op=Mult
                )
                nc.vector.tensor_tensor(
                    out=rhn[:], in0=rhn[:], in1=view3(ps_in), op=Add
                )
                nc.scalar.activation(n_sb[:], rhn[:], Tanh)
                nc.vector.tensor_tensor(out=t1[:], in0=h4[:], in1=n_sb[:], op=Sub)
                nc.vector.tensor_tensor(out=t1[:], in0=z_sb[:], in1=t1[:], op=Mult)
                nc.vector.tensor_tensor(out=h4[:], in0=n_sb[:], in1=t1[:], op=Add)
                nc.vector.tensor_tensor(
                    out=hacc4[:], in0=hacc4[:], in1=h4[:], op=Add
                )

        # ---- outputs: raw h-sum (x3*8) and final hiddens of the 64 nodes ----
        nc.sync.dma_start(out_hacc[:], hacc4[:])
        nc.sync.dma_start(out_hf[:], h4[:])


def _build_program():
    if "prog" in _PROG_CACHE:
        return _PROG_CACHE["prog"]
    import concourse.bacc as bacc
    import concourse.mybir as mybir
    import concourse.tile as tile

    f32 = mybir.dt.float32
    bf16 = mybir.dt.bfloat16

    nc = bacc.Bacc(
        "TRN2",
        target_bir_lowering=False,
        debug=False,
        enable_asserts=False,
        num_devices=NCORES,
        num_swdge_queues=2,
    )
    tokens = nc.dram_tensor(
        "tokens", [P, ARITY * 2 * (P5 // 32)], mybir.dt.int16, kind="ExternalInput"
    ).ap()
    embed = nc.dram_tensor("embed", [VOCAB, DIM], bf16, kind="ExternalInput").ap()
    wih_t = nc.dram_tensor("wih_t", [P, J, 9, P], bf16, kind="ExternalInput").ap()
    wih_s = nc.dram_tensor("wih_s", [P, J, 9, P], bf16, kind="ExternalInput").ap()
    whh_t = nc.dram_tensor("whh_t", [P, J, 9, P], bf16, kind="ExternalInput").ap()
    biases = nc.dram_tensor("biases", [P, 12], f32, kind="ExternalInput").ap()
    biases_mm = nc.dram_tensor("biases_mm", [3, 4, P], bf16, kind="ExternalInput").ap()
    onehot3 = nc.dram_tensor("onehot3", [3, 3, 512], bf16, kind="ExternalInput").ap()
    out_hacc = nc.dram_tensor("out_hacc", [P, J, P4], f32, kind="ExternalOutput").ap()
    out_hf = nc.dram_tensor("out_hf", [P, J, P4], bf16, kind="ExternalOutput").ap()

    with tile.TileContext(nc) as tc:
        _emit(
            tc,
            nc,
            (
                tokens,
                embed,
                wih_t,
                wih_s,
                whh_t,
                biases,
                biases_mm,
                onehot3,
                out_hacc,
                out_hf,
            ),
        )
    nc.compile()
    _PROG_CACHE["prog"] = nc
    return nc


def _retile_weights(w):
    # w: [1152, 384] -> lhsT tiles [128(k_part), 3(k), 9(m), 128(m_col)] bf16
    wt = np.ascontiguousarray(w.T)  # [384, 1152]
    wt = wt.reshape(J, P, 9, P).transpose(1, 0, 2, 3)
    return np.ascontiguousarray(wt).astype(BF16)


def _prep_bias(b_ih, b_hh):
    biases = np.zeros((P, 12), np.float32)
    comb = (b_ih + b_hh).reshape(9, P)
    biases[:, 0:6] = comb[0:6].T
    biases[:, 6:9] = b_hh.reshape(9, P)[6:9].T
    biases[:, 9:12] = b_ih.reshape(9, P)[6:9].T
    return biases


def _prep_bias_mm(b_ih, b_hh):
    # lhsT[k, ro, q] = bias[q, 3*ro + k]: the K=3 bias matmul against the
    # one-hot rhs yields out[q, (j, n)] = bias[q, 3*ro + j].
    b = _prep_bias(b_ih, b_hh)  # [128, 12] cols: r0..2 z0..2 hn0..2 in0..2
    out = b.T.reshape(4, 3, P).transpose(1, 0, 2)
    return np.ascontiguousarray(out).astype(BF16)


def _prep_onehot3():
    out = np.zeros((3, 3, 512), np.float32)
    for k in range(3):
        out[k, k, :] = 1.0
    return out.astype(BF16)


def _prep_tokens(tokens_core):
    # int16 gather indices: block (c, half) holds the 256 tokens of child c,
    # parents q = half*256 + i, wrapped into 16 partitions ([16, 16] with
    # position i at [i % 16, i // 16]) and replicated x8 across the
    # partition groups.
    tok = tokens_core.reshape(P5, ARITY).T.astype(np.int16)  # [8 child, 512]
    ncols = P5 // 32  # 16 columns per half-block
    out = np.empty((16, ARITY * 2 * ncols), np.int16)
    for c in range(ARITY):
        for half in range(2):
            blk = c * 2 + half
            seg = tok[c, half * 256 : (half + 1) * 256]
            out[:, blk * ncols : (blk + 1) * ncols] = seg.reshape(ncols, 16).T
    return np.ascontiguousarray(np.tile(out, (8, 1)))


def _gru_level(x_children, h0, w_ih, w_hh, b_ih, b_hh):
    # x_children: [A, N, D] in original child order; consumed reversed.
    h = h0
    acc = np.zeros_like(h)
    for t in range(ARITY):
        x_t = x_children[ARITY - 1 - t]
        gi = x_t @ w_ih.T + b_ih
        gh = h @ w_hh.T + b_hh
        i_r, i_z, i_n = np.split(gi, 3, axis=-1)
        h_r, h_z, h_n = np.split(gh, 3, axis=-1)
        r = 1.0 / (1.0 + np.exp(-(i_r + h_r)))
        z = 1.0 / (1.0 + np.exp(-(i_z + h_z)))
        n = np.tanh(i_n + r * h_n)
        h = (1.0 - z) * n + z * h
        acc += h
    return acc / ARITY, h


def kernel(leaf_tokens, embed_table, w_ih, w_hh, b_ih, b_hh):
    from concourse.bass_utils import run_bass_kernel_spmd

    leaf_tokens = np.asarray(leaf_tokens, np.int32)
    embed_table = np.asarray(embed_table, np.float32)
    w_ih = np.asarray(w_ih, np.float32)
    w_hh = np.asarray(w_hh, np.float32)
    b_ih = np.asarray(b_ih, np.float32)
    b_hh = np.asarray(b_hh, np.float32)

    nc = _build_program()

    embed_bf = embed_table.astype(BF16)
    wih_t = _retile_weights(w_ih)
    wih_s = _retile_weights(w_ih / ARITY)
    whh_t = _retile_weights(w_hh)
    biases = _prep_bias(b_ih, b_hh)
    biases_mm = _prep_bias_mm(b_ih, b_hh)
    onehot3 = _prep_onehot3()
    in_maps = []
    for core in range(NCORES):
        in_maps.append(
            {
                "tokens": _prep_tokens(
                    leaf_tokens[core * LEAVES_CORE : (core + 1) * LEAVES_CORE]
                ),
                "embed": embed_bf,
                "wih_t": wih_t,
                "wih_s": wih_s,
                "whh_t": whh_t,
                "biases": biases,
                "biases_mm": biases_mm,
                "onehot3": onehot3,
            }
        )
    res = run_bass_kernel_spmd(nc, in_maps, core_ids=list(range(NCORES)))

    # device tensors -> [core, 64 nodes, 384] with f = j*128 + p
    x3 = np.zeros((NCORES, P4, DIM), np.float64)
    h3 = np.zeros((NCORES, P4, DIM), np.float64)
    for core in range(NCORES):
        hacc = np.asarray(res.results[core]["out_hacc"], np.float64)  # [128,3,64]
        hf = np.asarray(res.results[core]["out_hf"], np.float64)
        x3[core] = (hacc / ARITY).transpose(1, 0, 2).reshape(DIM, P4).T
        h3[core] = hf.transpose(1, 0, 2).reshape(DIM, P4).T

    w_ih64 = w_ih.astype(np.float64)
    w_hh64 = w_hh.astype(np.float64)
    b_ih64 = b_ih.astype(np.float64)
    b_hh64 = b_hh.astype(np.float64)

    # level 3: per core, 8 parents x 8 children (batch over cores*parents)
    xc = x3.reshape(NCORES * ARITY, ARITY, DIM).transpose(1, 0, 2)  # [A, 64, D]
    h0 = h3.reshape(NCORES * ARITY, ARITY, DIM).mean(axis=1)
    x2, h2 = _gru_level(xc, h0, w_ih64, w_hh64, b_ih64, b_hh64)

    # level 2: per core, 1 parent x 8 children
    xc = x2.reshape(NCORES, ARITY, DIM).transpose(1, 0, 2)  # [A, 8, D]
    h0 = h2.reshape(NCORES, ARITY, DIM).mean(axis=1)
    x1, h1 = _gru_level(xc, h0, w_ih64, w_hh64, b_ih64, b_hh64)

    # root: 8 cores' outputs
    xc = x1.reshape(1, ARITY, DIM).transpose(1, 0, 2)  # [A, 1, D]
    h0 = h1.reshape(1, ARITY, DIM).mean(axis=1)
    out, _ = _gru_level(xc, h0, w_ih64, w_hh64, b_ih64, b_hh64)

    return out.astype(np.float32).reshape(1, 1, DIM)


# revision 26
# speedup vs baseline: 1.6384x; 1.1478x over previous
"""Tree-GRU (arity-8, depth-5) over embedded leaves on 8 TRN2 NeuronCores.

Sharding: data-parallel over subtrees. Each core takes 4096 contiguous leaves
and runs levels 5 and 4 of the tree locally (512 -> 64 parents). The last two
per-core levels (64 -> 8 -> 1) and the root are small latency-bound GRU
cascades (free dim <= 8) done on host in fp64 after gathering the per-core
level-4 outputs, extending the baseline's host-side root reduction.

Device layout is feature-transposed: tensors live as [128 part, 3 ktile, ...]
with feature f = 128*k + p, so the GRU matmuls contract the partition dim.

Embeddings arrive per GRU step: tokens are host-permuted child-major, each
child's 512 rows fetched by 4 indirect DMAs (leaf-major) and flipped
feature-major by 4 xbar transpose-DMAs on the HWDGE rings — no tensor-engine
transposes, no PSUM, and the first GRU matmul can start after ~2 gathers.

Level 512 keeps one PSUM bank per (role, jo) output tile at N=512; each step
emits gi matmuls of units j0/j1 ahead of any hh matmul so the tensor engine
holds ~3.8us of h-independent work to hide the previous step's gate chain.
Unit j1 owns 4 banks (double-buffered step to step); j0 and j2 share the
other 4, with j2's allocation waiting on j0's progressively-freed banks
behind hh j1. Biases ride the scalar-activation bias port. The per-step
output accumulator is kept child-major (vector engine) so the final step
writes level 4's input directly as a fused raw-sum add; the 1/8 output-mean
scale is folded into a pre-scaled copy of W_ih used by level 4. Level 64
injects biases into PSUM via a K=3 one-hot matmul (the only start=True
write), collapsing the gate chain to jo-spanning instructions.
"""

import numpy as np
import ml_dtypes

ARITY = 8
DIM = 384
VOCAB = 32000
NCORES = 8
P = 128
J = 3  # DIM // 128 feature tiles
N_LEAVES = 32768
LEAVES_CORE = N_LEAVES // NCORES  # 4096
P5 = LEAVES_CORE // ARITY  # 512 level-5 parents per core
P4 = P5 // ARITY  # 64 level-4 parents per core
GT = P5 // P  # 4 gather tiles per child

BF16 = ml_dtypes.bfloat16

_PROG_CACHE = {}


def _emit(tc, nc, aps):
    import concourse.mybir as mybir
    import concourse.bass as bass

    f32 = mybir.dt.float32
    bf16 = mybir.dt.bfloat16
    Sig = mybir.ActivationFunctionType.Sigmoid
    Tanh = mybir.ActivationFunctionType.Tanh
    Add = mybir.AluOpType.add
    Sub = mybir.AluOpType.subtract
    Mult = mybir.AluOpType.mult

    tokens, embed, wih_t, biases, biases_mm, out_hacc, out_hf = aps

    from contextlib import ExitStack

    with ExitStack() as ctx:
        const = ctx.enter_context(tc.tile_pool(name="const", bufs=1))
        xpool = ctx.enter_context(tc.tile_pool(name="xpool", bufs=1))
        state = ctx.enter_context(tc.tile_pool(name="state", bufs=1))
        gates = ctx.enter_context(tc.tile_pool(name="gates", bufs=4))
        pspool = ctx.enter_context(tc.tile_pool(name="pspool", bufs=4, space="PSUM"))
        pspool2 = ctx.enter_context(tc.tile_pool(name="pspool2", bufs=4, space="PSUM"))

        # ---- index tile first, then feature-major embedding gathers ----
        idx_sb = const.tile([P, ARITY * (P5 // 16)], mybir.dt.int16)
        nc.sync.dma_start(idx_sb[:], tokens[:])

        wpack_sb = const.tile([P, 3, J, 9, P], bf16)
        wih_sb = wpack_sb[:, 0]
        whh_sb = wpack_sb[:, 1]
        wih_s_sb = wpack_sb[:, 2]
        bias_sb = const.tile([P, 12], f32)
        bpack_sb = const.tile([3, 4 * P + 3 * 512], bf16)
        bias3_sb = bpack_sb[:, : 4 * P].rearrange("k (r p) -> k r p", r=4)
        onehot3_sb = bpack_sb[:, 4 * P :].rearrange("k (j n) -> k j n", j=3)
        nc.sync.dma_start(wpack_sb[:], wih_t[:])
        nc.sync.dma_start(bias_sb[:], biases[:])
        nc.sync.dma_start(bpack_sb[:], biases_mm[:])

        # x5[p, child, j, q]: one feature-major dma_gather per child (the
        # gather ucode transposes at the xbar; out must be contiguous)
        x5 = xpool.tile([P, ARITY, J, P5], bf16, name="x5", tag="x5")
        ncols = P5 // 16
        for t in range(ARITY):
            c = ARITY - 1 - t  # children consumed in reverse: child 7 first
            nc.gpsimd.dma_gather(
                x5[:, c],
                embed[:],
                idx_sb[:, c * ncols : (c + 1) * ncols],
                P5,
                P5,
                DIM,
                transpose=True,
                queue_num=t % 2,
            )

        x4 = xpool.tile([P, ARITY, J, P4], bf16, name="x4", tag="x4")

        def psum_tile(jo):
            # 8 banks for 12 role-tiles per step: unit j1 owns pspool (reuse
            # waits on the previous step's j1 gates); j0/j2 share pspool2 —
            # j2 waits on same-step j0 gates (freed progressively under hh
            # j1), j0 on the previous step's j2 gates. All waits point at
            # strictly earlier FIFO positions: no deadlock.
            if jo == 1:
                return pspool.tile([P, 512], f32, name="ps", tag="ps")
            return pspool2.tile([P, 512], f32, name="ps2", tag="ps2")

        # =================== level 5: 512 parents, leaf children ===================
        h5 = state.tile([P, J, P5], bf16, name="h5", tag="h5")
        hacc5 = state.tile([P, J, ARITY, P4], f32, name="hacc5", tag="hacc5")
        nc.gpsimd.memset(hacc5[:], 0.0)
        csum5 = state.tile([P, J, P4], f32, name="csum5", tag="csum5")

        with nc.named_scope("level_512"):
            for t in range(ARITY):
                c = ARITY - 1 - t
                leaf0 = t == 0

                ps_r = [None] * J
                ps_z = [None] * J
                ps_in = [None] * J
                ps_hn = [None] * J

                def emit_gi(jo):
                    ps_r[jo] = psum_tile(jo)
                    ps_z[jo] = psum_tile(jo)
                    ps_in[jo] = psum_tile(jo)
                    if not leaf0:
                        ps_hn[jo] = psum_tile(jo)
                    for ps, moff in ((ps_r[jo], 0), (ps_z[jo], 3), (ps_in[jo], 6)):
                        for ji in range(J):
                            nc.tensor.matmul(
                                ps[:, :P5],
                                wih_sb[:, ji, moff + jo, :],
                                x5[:, c, ji, :],
                                start=(ji == 0),
                                stop=(ji == 2 and (moff == 6 or leaf0)),
                            )

                def emit_hh(jo):
                    if leaf0:
                        return
                    for ps, moff in ((ps_r[jo], 0), (ps_z[jo], 3), (ps_hn[jo], 6)):
                        for ji in range(J):
                            nc.tensor.matmul(
                                ps[:, :P5],
                                whh_sb[:, ji, moff + jo, :],
                                h5[:, ji, :],
                                start=(ji == 0 and moff == 6),
                                stop=(ji == 2),
                            )

                emit_gi(0)
                emit_gi(1)
                emit_hh(0)
                emit_hh(1)
                emit_gi(2)
                emit_hh(2)

                for jo in range(J):
                    r_sb = gates.tile([P, P5], bf16, name="r_sb", tag="r_sb")
                    z_sb = gates.tile([P, P5], bf16, name="z_sb", tag="z_sb")
                    n_sb = gates.tile([P, P5], bf16, name="n_sb", tag="n_sb")
                    rhn = gates.tile([P, P5], f32, name="rhn", tag="rhn")
                    t1 = gates.tile([P, P5], bf16, name="t1", tag="t1")

                    nc.scalar.activation(
                        r_sb[:], ps_r[jo][:, :P5], Sig, bias=bias_sb[:, jo : jo + 1]
                    )
                    nc.scalar.activation(
                        z_sb[:], ps_z[jo][:, :P5], Sig, bias=bias_sb[:, 3 + jo : 4 + jo]
                    )
                    if leaf0:
                        nc.vector.tensor_scalar_mul(
                            rhn[:], r_sb[:], bias_sb[:, 6 + jo : 7 + jo]
                        )
                    else:
                        nc.vector.scalar_tensor_tensor(
                            out=rhn[:],
                            in0=ps_hn[jo][:, :P5],
                            scalar=bias_sb[:, 6 + jo : 7 + jo],
                            in1=r_sb[:],
                            op0=Add,
                            op1=Mult,
                        )
                    nc.vector.tensor_tensor(
                        out=rhn[:], in0=rhn[:], in1=ps_in[jo][:, :P5], op=Add
                    )
                    nc.scalar.activation(
                        n_sb[:], rhn[:], Tanh, bias=bias_sb[:, 9 + jo : 10 + jo]
                    )
                    hsl = h5[:, jo, :]
                    if leaf0:
                        nc.vector.tensor_tensor(out=t1[:], in0=z_sb[:], in1=n_sb[:], op=Mult)
                        nc.vector.tensor_tensor(out=hsl, in0=n_sb[:], in1=t1[:], op=Sub)
                    else:
                        nc.vector.tensor_tensor(out=t1[:], in0=hsl, in1=n_sb[:], op=Sub)
                        nc.vector.tensor_tensor(out=t1[:], in0=z_sb[:], in1=t1[:], op=Mult)
                        nc.vector.tensor_tensor(out=hsl, in0=n_sb[:], in1=t1[:], op=Add)

                # output accumulation after the chain ops so it never delays
                # the next step's recurrent matmuls
                for jo in range(J):
                    hsl = h5[:, jo, :]
                    hperm = hsl.rearrange("p (q c) -> p c q", c=ARITY)
                    if t == ARITY - 1:
                        nc.vector.tensor_reduce(
                            out=csum5[:, jo, :],
                            in_=hsl.rearrange("p (q c) -> p q c", c=ARITY),
                            axis=mybir.AxisListType.X,
                            op=Add,
                        )
                        nc.vector.tensor_tensor(
                            out=x4[:, :, jo, :],
                            in0=hacc5[:, jo],
                            in1=hperm,
                            op=Add,
                        )
                    else:
                        nc.vector.tensor_tensor(
                            out=hacc5[:, jo], in0=hacc5[:, jo], in1=hperm, op=Add
                        )

        # =================== level 4: 64 parents ===================
        h4 = state.tile([P, J, P4], bf16, name="h4", tag="h4")
        nc.scalar.mul(h4[:], csum5[:], 1.0 / ARITY)
        hacc4 = state.tile([P, J, P4], f32, name="hacc4", tag="hacc4")
        nc.gpsimd.memset(hacc4[:], 0.0)
        N3 = J * P4  # 192

        with nc.named_scope("level_64"):
            for t in range(ARITY):
                c = ARITY - 1 - t
                pool = pspool if t % 2 == 0 else pspool2
                tag = "ps" if t % 2 == 0 else "ps2"
                ps_r, ps_z, ps_hn, ps_in = (
                    pool.tile([P, 512], f32, name=tag, tag=tag) for _ in range(4)
                )

                def view3(pst):
                    return pst[:, :N3].rearrange("p (j n) -> p j n", j=3)

                # bias matmul: the only start=True write, covering each tile
                for pst, ro in ((ps_r, 0), (ps_z, 1), (ps_hn, 2), (ps_in, 3)):
                    nc.tensor.matmul(
                        pst[:, :N3],
                        bias3_sb[:, ro, :],
                        onehot3_sb[:, :, :P4],
                        start=True,
                        stop=False,
                    )
                for pst, moff in ((ps_r, 0), (ps_z, 3), (ps_in, 6)):
                    for jo in range(J):
                        for ji in range(J):
                            nc.tensor.matmul(
                                pst[:, jo * P4 : (jo + 1) * P4],
                                wih_s_sb[:, ji, moff + jo, :],
                                x4[:, c, ji, :],
                                start=False,
                                stop=(moff == 6 and jo == 2 and ji == 2),
                            )
                for pst, moff in ((ps_r, 0), (ps_z, 3), (ps_hn, 6)):
                    for jo in range(J):
                        for ji in range(J):
                            nc.tensor.matmul(
                                pst[:, jo * P4 : (jo + 1) * P4],
                                whh_sb[:, ji, moff + jo, :],
                                h4[:, ji, :],
                                start=False,
                                stop=(jo == 2 and ji == 2),
                            )

                r_sb = gates.tile([P, J, P4], bf16, name="r4", tag="r4")
                z_sb = gates.tile([P, J, P4], bf16, name="z4", tag="z4")
                n_sb = gates.tile([P, J, P4], bf16, name="n4", tag="n4")
                rhn = gates.tile([P, J, P4], f32, name="rhn4", tag="rhn4")
                t1 = gates.tile([P, J, P4], bf16, name="t14", tag="t14")

                nc.scalar.activation(r_sb[:], view3(ps_r), Sig)
                nc.scalar.activation(z_sb[:], view3(ps_z), Sig)
                nc.vector.tensor_tensor(
                    out=rhn[:], in0=view3(ps_hn), in1=r_sb[:], op=Mult
                )
                nc.vector.tensor_tensor(
                    out=rhn[:], in0=rhn[:], in1=view3(ps_in), op=Add
                )
                nc.scalar.activation(n_sb[:], rhn[:], Tanh)
                nc.vector.tensor_tensor(out=t1[:], in0=h4[:], in1=n_sb[:], op=Sub)
                nc.vector.tensor_tensor(out=t1[:], in0=z_sb[:], in1=t1[:], op=Mult)
                nc.vector.tensor_tensor(out=h4[:], in0=n_sb[:], in1=t1[:], op=Add)
                nc.vector.tensor_tensor(
                    out=hacc4[:], in0=hacc4[:], in1=h4[:], op=Add
                )

        # ---- outputs: raw h-sum (x3*8) and final hiddens of the 64 nodes ----
        nc.sync.dma_start(out_hacc[:], hacc4[:])
        nc.sync.dma_start(out_hf[:], h4[:])


def _build_program():
    if "prog" in _PROG_CACHE:
        return _PROG_CACHE["prog"]
    import concourse.bacc as bacc
    import concourse.mybir as mybir
    import concourse.tile as tile

    f32 = mybir.dt.float32
    bf16 = mybir.dt.bfloat16

    nc = bacc.Bacc(
        "TRN2",
        target_bir_lowering=False,
        debug=False,
        enable_asserts=False,
        num_devices=NCORES,
        num_swdge_queues=2,
    )
    tokens = nc.dram_tensor(
        "tokens", [P, ARITY * (P5 // 16)], mybir.dt.int16, kind="ExternalInput"
    ).ap()
    embed = nc.dram_tensor("embed", [VOCAB, DIM], bf16, kind="ExternalInput").ap()
    wpack = nc.dram_tensor("wpack", [P, 3, J, 9, P], bf16, kind="ExternalInput").ap()
    biases = nc.dram_tensor("biases", [P, 12], f32, kind="ExternalInput").ap()
    bpack = nc.dram_tensor(
        "bpack", [3, 4 * P + 3 * 512], bf16, kind="ExternalInput"
    ).ap()
    out_hacc = nc.dram_tensor("out_hacc", [P, J, P4], f32, kind="ExternalOutput").ap()
    out_hf = nc.dram_tensor("out_hf", [P, J, P4], bf16, kind="ExternalOutput").ap()

    with tile.TileContext(nc) as tc:
        _emit(tc, nc, (tokens, embed, wpack, biases, bpack, out_hacc, out_hf))
    nc.compile()
    _PROG_CACHE["prog"] = nc
    return nc


def _retile_weights(w):
    # w: [1152, 384] -> lhsT tiles [128(k_part), 3(k), 9(m), 128(m_col)] bf16
    wt = np.ascontiguousarray(w.T)  # [384, 1152]
    wt = wt.reshape(J, P, 9, P).transpose(1, 0, 2, 3)
    return np.ascontiguousarray(wt).astype(BF16)


def _prep_bias(b_ih, b_hh):
    biases = np.zeros((P, 12), np.float32)
    comb = (b_ih + b_hh).reshape(9, P)
    biases[:, 0:6] = comb[0:6].T
    biases[:, 6:9] = b_hh.reshape(9, P)[6:9].T
    biases[:, 9:12] = b_ih.reshape(9, P)[6:9].T
    return biases


def _prep_bias_mm(b_ih, b_hh):
    # lhsT[k, ro, q] = bias[q, 3*ro + k]: the K=3 bias matmul against the
    # one-hot rhs yields out[q, (j, n)] = bias[q, 3*ro + j].
    b = _prep_bias(b_ih, b_hh)  # [128, 12] cols: r0..2 z0..2 hn0..2 in0..2
    out = b.T.reshape(4, 3, P).transpose(1, 0, 2)
    return np.ascontiguousarray(out).astype(BF16)


def _prep_onehot3():
    out = np.zeros((3, 3, 512), np.float32)
    for k in range(3):
        out[k, k, :] = 1.0
    return out.astype(BF16)


def _prep_tokens(tokens_core):
    # int16 gather indices: block c holds the 512 tokens of child c, wrapped
    # into 16 partitions (position i at [i % 16, i // 16]) and replicated x8
    # across the partition groups.
    tok = tokens_core.reshape(P5, ARITY).T.astype(np.int16)  # [8 child, 512]
    ncols = P5 // 16
    out = np.empty((16, ARITY * ncols), np.int16)
    for c in range(ARITY):
        out[:, c * ncols : (c + 1) * ncols] = tok[c].reshape(ncols, 16).T
    return np.ascontiguousarray(np.tile(out, (8, 1)))


def _gru_level(x_children, h0, w_ih, w_hh, b_ih, b_hh):
    # x_children: [A, N, D] in original child order; consumed reversed.
    h = h0
    acc = np.zeros_like(h)
    for t in range(ARITY):
        x_t = x_children[ARITY - 1 - t]
        gi = x_t @ w_ih.T + b_ih
        gh = h @ w_hh.T + b_hh
        i_r, i_z, i_n = np.split(gi, 3, axis=-1)
        h_r, h_z, h_n = np.split(gh, 3, axis=-1)
        r = 1.0 / (1.0 + np.exp(-(i_r + h_r)))
        z = 1.0 / (1.0 + np.exp(-(i_z + h_z)))
        n = np.tanh(i_n + r * h_n)
        h = (1.0 - z) * n + z * h
        acc += h
    return acc / ARITY, h


def kernel(leaf_tokens, embed_table, w_ih, w_hh, b_ih, b_hh):
    from concourse.bass_utils import run_bass_kernel_spmd

    leaf_tokens = np.asarray(leaf_tokens, np.int32)
    embed_table = np.asarray(embed_table, np.float32)
    w_ih = np.asarray(w_ih, np.float32)
    w_hh = np.asarray(w_hh, np.float32)
    b_ih = np.asarray(b_ih, np.float32)
    b_hh = np.asarray(b_hh, np.float32)

    nc = _build_program()

    embed_bf = embed_table.astype(BF16)
    wpack = np.ascontiguousarray(
        np.stack(
            [
                _retile_weights(w_ih),
                _retile_weights(w_hh),
                _retile_weights(w_ih / ARITY),
            ],
            axis=1,
        )
    )
    biases = _prep_bias(b_ih, b_hh)
    bpack = np.ascontiguousarray(
        np.concatenate(
            [
                _prep_bias_mm(b_ih, b_hh).reshape(3, 4 * P),
                _prep_onehot3().reshape(3, 3 * 512),
            ],
            axis=1,
        )
    )
    in_maps = []
    for core in range(NCORES):
        in_maps.append(
            {
                "tokens": _prep_tokens(
                    leaf_tokens[core * LEAVES_CORE : (core + 1) * LEAVES_CORE]
                ),
                "embed": embed_bf,
                "wpack": wpack,
                "biases": biases,
                "bpack": bpack,
            }
        )
    res = run_bass_kernel_spmd(nc, in_maps, core_ids=list(range(NCORES)))

    # device tensors -> [core, 64 nodes, 384] with f = j*128 + p
    x3 = np.zeros((NCORES, P4, DIM), np.float64)
    h3 = np.zeros((NCORES, P4, DIM), np.float64)
    for core in range(NCORES):
        hacc = np.asarray(res.results[core]["out_hacc"], np.float64)  # [128,3,64]
        hf = np.asarray(res.results[core]["out_hf"], np.float64)
        x3[core] = (hacc / ARITY).transpose(1, 0, 2).reshape(DIM, P4).T
        h3[core] = hf.transpose(1, 0, 2).reshape(DIM, P4).T

    w_ih64 = w_ih.astype(np.float64)
    w_hh64 = w_hh.astype(np.float64)
    b_ih64 = b_ih.astype(np.float64)
    b_hh64 = b_hh.astype(np.float64)

    # level 3: per core, 8 parents x 8 children (batch over cores*parents)
    xc = x3.reshape(NCORES * ARITY, ARITY, DIM).transpose(1, 0, 2)  # [A, 64, D]
    h0 = h3.reshape(NCORES * ARITY, ARITY, DIM).mean(axis=1)
    x2, h2 = _gru_level(xc, h0, w_ih64, w_hh64, b_ih64, b_hh64)

    # level 2: per core, 1 parent x 8 children
    xc = x2.reshape(NCORES, ARITY, DIM).transpose(1, 0, 2)  # [A, 8, D]
    h0 = h2.reshape(NCORES, ARITY, DIM).mean(axis=1)
    x1, h1 = _gru_level(xc, h0, w_ih64, w_hh64, b_ih64, b_hh64)

    # root: 8 cores' outputs
    xc = x1.reshape(1, ARITY, DIM).transpose(1, 0, 2)  # [A, 1, D]
    h0 = h1.reshape(1, ARITY, DIM).mean(axis=1)
    out, _ = _gru_level(xc, h0, w_ih64, w_hh64, b_ih64, b_hh64)

    return out.astype(np.float32).reshape(1, 1, DIM)
